# revision 10
# baseline (speedup 1.0000x reference)
"""DistMult scoring kernel for Trainium2 (8 NeuronCores, Bass/Tile).

reference computation:
    rel = rel_embeds[rel_ids]                      # [B, D] gather
    scores = sum(head * rel * tail, axis=-1)       # [B]
    pos = min(scores[:n_pos], upper_bound)
    neg = max(scores[n_pos:], lower_bound)
    out = sigmoid(concat(pos, neg))

Sharding: data-parallel over B. Core c owns rows [c*65536, (c+1)*65536).
Within a core, local row r maps to (partition p, column t) with r = p*T + t,
so every stream DMA is contiguous per partition and the final [128, 512]
score tile stores with one contiguous DMA.

The pos/neg split falls on a core boundary (131072 = 2 * 65536), handled
data-parallel by feeding cores +/-inf padded bounds:
    out = sigmoid(max(min(scores, ub), lb))

Design notes (vs. the earlier pair-table version):
  * h / t / rel table stream as fp8_e4m3 (l2 err ~1e-2, under the 2e-2 gate).
  * rel rows are fetched with one dma_gather per chunk (4096 int16 indices,
    one SWDGE instruction) instead of 32 indirect_dma_starts, cutting GpSimd
    descriptor-generation time ~8x and HBM gather traffic 4x (256B fp8 rows
    from the 500-row table instead of 1KiB bf16 pair rows).
  * per chunk the DVE work is 3 large instructions: q = h*t, rq = q*rel,
    and a segmented tensor_reduce [128, G, 256] -> [128, G], avoiding the
    512 tiny accumulate ops that dominated Vector time before.
"""

import sys

for _p in ("/opt/trn_rl_repo",):
    if _p not in sys.path:
        sys.path.insert(0, _p)

import numpy as np

import concourse.bacc as bacc
import concourse.bass as bass
import concourse.mybir as mybir
import concourse.tile as tile
from concourse.bass_utils import run_bass_kernel_spmd

N_POS = 131072
N_NEG = 393216
B = N_POS + N_NEG  # 524288
D = 256
NUM_REL = 500
NCORES = 8
ROWS = B // NCORES  # 65536 rows per core
P = 128
T = ROWS // P  # 512 tiles of 128 rows; local row = p*T + t
G = 32  # tile columns per chunk
NCH = T // G  # 16 chunks
CHUNK_IDX = P * G  # 4096 gather indices per chunk
IDX_COLS = ROWS // 16  # 4096 int16 columns in the idx tensor

# stream dtype for head/tail ("f32", "bf16", or "f8")
STREAM_DT = "f8"
# dtype for the rel table gather
TABLE_DT = "f8"
# "dma_gather" (one SWDGE gather instr per chunk, int16 idxs) or
# "indirect" (indirect_dma_start with a [128, G] int32 offset AP per chunk) or
# "indirect1" (G indirect_dma_starts per chunk, [128, 1] offsets like baseline)
GATHER_MODE = "indirect1"


def _dt(tag: str):
    return {
        "f32": mybir.dt.float32,
        "bf16": mybir.dt.bfloat16,
        "f8": mybir.dt.float8e4,
    }[tag]


def _np_dt(tag: str):
    import ml_dtypes

    return {
        "f32": np.float32,
        "bf16": ml_dtypes.bfloat16,
        "f8": ml_dtypes.float8_e4m3,
    }[tag]


def build_program(stream_dt: str = STREAM_DT, table_dt: str = TABLE_DT):
    sdt = _dt(stream_dt)
    tdt = _dt(table_dt)
    f32 = mybir.dt.float32
    bf16 = mybir.dt.bfloat16
    i16 = mybir.dt.int16
    i32 = mybir.dt.int32
    mult = mybir.AluOpType.mult
    add = mybir.AluOpType.add

    nc = bacc.Bacc(
        "TRN2", target_bir_lowering=False, debug=False, num_devices=NCORES
    )
    h = nc.declare_dram_parameter("h", [ROWS, D], sdt, isOutput=False)
    t_ = nc.declare_dram_parameter("t", [ROWS, D], sdt, isOutput=False)
    if GATHER_MODE == "dma_gather":
        # gather indices, [128, ROWS//16] int16; row j<16 column s holds the
        # rel id for gather element i = s*16 + j (rows 16..127 replicate 0..15)
        idx = nc.declare_dram_parameter("idx", [P, IDX_COLS], i16, isOutput=False)
    else:
        # idx[p, t] = rel id of local row p*T + t, int32
        idx = nc.declare_dram_parameter("idx", [P, T], i32, isOutput=False)
    ub = nc.declare_dram_parameter("ub", [ROWS], f32, isOutput=False)
    lb = nc.declare_dram_parameter("lb", [ROWS], f32, isOutput=False)
    table = nc.declare_dram_parameter("table", [NUM_REL, D], tdt, isOutput=False)
    out = nc.declare_dram_parameter("out", [ROWS], f32, isOutput=True)

    h_v = h[:].rearrange("(p t) d -> p t d", p=P)
    t_v = t_[:].rearrange("(p t) d -> p t d", p=P)
    ub_v = ub[:].rearrange("(p t) -> p t", p=P)
    lb_v = lb[:].rearrange("(p t) -> p t", p=P)
    out_v = out[:].rearrange("(p t) -> p t", p=P)

    with tile.TileContext(nc) as tc:
        with (
            tc.tile_pool(name="io", bufs=1) as io_pool,
            tc.tile_pool(name="stream", bufs=3) as spool,
            tc.tile_pool(name="rpool", bufs=3) as rpool,
            tc.tile_pool(name="qpool", bufs=2) as qpool,
        ):
            if GATHER_MODE == "dma_gather":
                idxs_all = io_pool.tile([P, IDX_COLS], i16)
            else:
                idxs_all = io_pool.tile([P, T], i32)
            nc.sync.dma_start(out=idxs_all[:], in_=idx[:])
            scores = io_pool.tile([P, T], f32)

            for c in range(NCH):
                t0 = c * G
                htile = spool.tile([P, G * D], sdt, tag="h")
                ttile = spool.tile([P, G * D], sdt, tag="t")
                rtile = rpool.tile([P, G * D], tdt, tag="r")
                nc.sync.dma_start(
                    out=htile[:].rearrange("p (g d) -> p g d", g=G),
                    in_=h_v[:, t0 : t0 + G, :],
                )
                nc.sync.dma_start(
                    out=ttile[:].rearrange("p (g d) -> p g d", g=G),
                    in_=t_v[:, t0 : t0 + G, :],
                )
                if GATHER_MODE == "dma_gather":
                    c0 = c * (CHUNK_IDX // 16)
                    nc.gpsimd.dma_gather(
                        out_ap=rtile[:].rearrange("p (g d) -> p g d", g=G),
                        in_ap=table[:],
                        idxs_ap=idxs_all[:, c0 : c0 + CHUNK_IDX // 16],
                        num_idxs=CHUNK_IDX,
                        num_idxs_reg=CHUNK_IDX,
                        elem_size=D,
                    )
                elif GATHER_MODE == "indirect":
                    nc.gpsimd.indirect_dma_start(
                        out=rtile[:].rearrange("p (g d) -> p g d", g=G),
                        out_offset=None,
                        in_=table[:],
                        in_offset=bass.IndirectOffsetOnAxis(
                            ap=idxs_all[:, t0 : t0 + G], axis=0
                        ),
                    )
                else:
                    for gi in range(G):
                        nc.gpsimd.indirect_dma_start(
                            out=rtile[:, gi * D : (gi + 1) * D],
                            out_offset=None,
                            in_=table[:],
                            in_offset=bass.IndirectOffsetOnAxis(
                                ap=idxs_all[:, t0 + gi : t0 + gi + 1], axis=0
                            ),
                        )
                q = qpool.tile([P, G * D], bf16, tag="q")
                rq = qpool.tile([P, G * D], bf16, tag="rq")
                nc.vector.tensor_tensor(
                    out=q[:], in0=htile[:], in1=ttile[:], op=mult
                )
                nc.vector.tensor_tensor(
                    out=rq[:], in0=q[:], in1=rtile[:], op=mult
                )
                nc.vector.tensor_reduce(
                    out=scores[:, t0 : t0 + G],
                    in_=rq[:].rearrange("p (g d) -> p g d", g=G),
                    axis=mybir.AxisListType.X,
                    op=add,
                )

            # tail: clamp + sigmoid + store
            ubt = io_pool.tile([P, T], f32)
            lbt = io_pool.tile([P, T], f32)
            nc.sync.dma_start(out=ubt[:], in_=ub_v)
            nc.sync.dma_start(out=lbt[:], in_=lb_v)
            clip1 = io_pool.tile([P, T], f32)
            clip2 = io_pool.tile([P, T], f32)
            nc.vector.tensor_tensor(
                out=clip1[:], in0=scores[:], in1=ubt[:], op=mybir.AluOpType.min
            )
            nc.vector.tensor_tensor(
                out=clip2[:], in0=clip1[:], in1=lbt[:], op=mybir.AluOpType.max
            )
            sig = io_pool.tile([P, T], f32)
            nc.scalar.activation(
                out=sig[:], in_=clip2[:], func=mybir.ActivationFunctionType.Sigmoid
            )
            nc.sync.dma_start(out=out_v, in_=sig[:])

    nc.compile()
    return nc


def make_in_maps(inputs: dict, stream_dt: str = STREAM_DT, table_dt: str = TABLE_DT):
    np_sdt = _np_dt(stream_dt)
    np_tdt = _np_dt(table_dt)

    head = np.asarray(inputs["head_embeds"], dtype=np.float32).astype(np_sdt)
    tail = np.asarray(inputs["tail_embeds"], dtype=np.float32).astype(np_sdt)
    rel_ids = np.asarray(inputs["rel_ids"]).astype(np.int32)
    lower = np.asarray(inputs["lower_bound"], dtype=np.float32)
    upper = np.asarray(inputs["upper_bound"], dtype=np.float32)
    table = np.asarray(inputs["rel_embeds"], dtype=np.float32).astype(np_tdt)

    pos_inf = np.full(ROWS, np.inf, dtype=np.float32)
    neg_inf = np.full(ROWS, -np.inf, dtype=np.float32)

    in_maps = []
    for c in range(NCORES):
        lo = c * ROWS
        hi = lo + ROWS
        if hi <= N_POS:
            ub_c = upper[lo:hi]
            lb_c = neg_inf
        else:
            assert lo >= N_POS
            ub_c = pos_inf
            lb_c = lower[lo - N_POS : hi - N_POS]
        # local row r = p*T + t
        rel_local = rel_ids[lo:hi].reshape(P, T)
        if GATHER_MODE == "dma_gather":
            # gather element i of chunk c is row (p = i%128, t = c*G + i//128),
            # stored at idx[i%16, c*256 + i//16]
            blocks = []
            for ch in range(NCH):
                flat = rel_local[:, ch * G : (ch + 1) * G].T.reshape(-1)
                blocks.append(flat.reshape(-1, 16).T)  # [16, 256]
            idx16 = np.hstack(blocks)  # [16, IDX_COLS]
            idx_full = np.tile(idx16, (P // 16, 1)).astype(np.int16)
        else:
            idx_full = rel_local.astype(np.int32)  # [128, T]
        in_maps.append(
            {
                "h": np.ascontiguousarray(head[lo:hi]),
                "t": np.ascontiguousarray(tail[lo:hi]),
                "idx": np.ascontiguousarray(idx_full),
                "ub": np.ascontiguousarray(ub_c),
                "lb": np.ascontiguousarray(lb_c),
                "table": table,
            }
        )
    return in_maps


def kernel(**inputs) -> np.ndarray:
    nc = build_program(STREAM_DT, TABLE_DT)
    in_maps = make_in_maps(inputs, STREAM_DT, TABLE_DT)
    res = run_bass_kernel_spmd(nc, in_maps, list(range(NCORES)))
    return np.concatenate([res.results[c]["out"] for c in range(NCORES)])


# revision 15
# speedup vs baseline: 1.3085x; 1.3085x over previous
"""DistMult scoring kernel for Trainium2 (8 NeuronCores, Bass/Tile).

reference computation:
    rel = rel_embeds[rel_ids]                      # [B, D] gather
    scores = sum(head * rel * tail, axis=-1)       # [B]
    pos = min(scores[:n_pos], upper_bound)
    neg = max(scores[n_pos:], lower_bound)
    out = sigmoid(concat(pos, neg))

Sharding: data-parallel over B. Core c owns rows [c*65536, (c+1)*65536).
Within a core, local row r maps to (partition p, column t) with r = p*T + t,
so every stream DMA is contiguous per partition and the final [128, 512]
score tile stores with one contiguous DMA.

The pos/neg split falls on a core boundary (131072 = 2 * 65536), handled
data-parallel by feeding cores +/-inf padded bounds:
    out = sigmoid(max(min(scores, ub), lb))

Design notes (vs. the earlier pair-table version):
  * h / t / rel table stream as fp8_e4m3 (l2 err ~1e-2, under the 2e-2 gate).
  * rel rows are fetched with one dma_gather per chunk (4096 int16 indices,
    one SWDGE instruction) instead of 32 indirect_dma_starts, cutting GpSimd
    descriptor-generation time ~8x and HBM gather traffic 4x (256B fp8 rows
    from the 500-row table instead of 1KiB bf16 pair rows).
  * per chunk the DVE work is 3 large instructions: q = h*t, rq = q*rel,
    and a segmented tensor_reduce [128, G, 256] -> [128, G], avoiding the
    512 tiny accumulate ops that dominated Vector time before.
"""

import sys

for _p in ("/opt/trn_rl_repo",):
    if _p not in sys.path:
        sys.path.insert(0, _p)

import numpy as np

import concourse.bacc as bacc
import concourse.bass as bass
import concourse.mybir as mybir
import concourse.tile as tile
from concourse.bass_utils import run_bass_kernel_spmd

N_POS = 131072
N_NEG = 393216
B = N_POS + N_NEG  # 524288
D = 256
NUM_REL = 500
NCORES = 8
ROWS = B // NCORES  # 65536 rows per core
P = 128
T = ROWS // P  # 512 tiles of 128 rows; local row = p*T + t
G = 16  # tile columns per chunk (stream DMA / DVE granularity)
NCH = T // G  # chunks per core
GB = 8  # tile columns per dma_gather: 1024 idxs -> 65 descs/lane. The SWDGE
# ring fits 128 descs/lane (2048-idx gathers fail on HW with INTERNAL).
NSUB = G // GB  # dma_gather calls per chunk
GATHER_IDX = P * GB  # 1024 indices per dma_gather
IDX_COLS = ROWS // 16  # int16 columns in the idx tensor

# stream dtype for head/tail ("f32", "bf16", or "f8")
STREAM_DT = "f8"
# dtype for the rel table gather
TABLE_DT = "f8"
# "dma_gather" (one SWDGE gather instr per chunk, int16 idxs) or
# "indirect" (indirect_dma_start with a [128, G] int32 offset AP per chunk;
#             BROKEN on HW - the DGE ucode only honors partition 0's offsets) or
# "indirect1" (G indirect_dma_starts per chunk, [128, 1] offsets like baseline)
GATHER_MODE = "dma_gather"


def _dt(tag: str):
    return {
        "f32": mybir.dt.float32,
        "bf16": mybir.dt.bfloat16,
        "f8": mybir.dt.float8e4,
    }[tag]


def _np_dt(tag: str):
    import ml_dtypes

    return {
        "f32": np.float32,
        "bf16": ml_dtypes.bfloat16,
        "f8": ml_dtypes.float8_e4m3,
    }[tag]


def build_program(stream_dt: str = STREAM_DT, table_dt: str = TABLE_DT):
    sdt = _dt(stream_dt)
    tdt = _dt(table_dt)
    f32 = mybir.dt.float32
    bf16 = mybir.dt.bfloat16
    i16 = mybir.dt.int16
    i32 = mybir.dt.int32
    mult = mybir.AluOpType.mult
    add = mybir.AluOpType.add

    nc = bacc.Bacc(
        "TRN2", target_bir_lowering=False, debug=False, num_devices=NCORES
    )
    h = nc.declare_dram_parameter("h", [ROWS, D], sdt, isOutput=False)
    t_ = nc.declare_dram_parameter("t", [ROWS, D], sdt, isOutput=False)
    if GATHER_MODE == "dma_gather":
        # gather indices, [128, ROWS//16] int16; row j<16 column s holds the
        # rel id for gather element i = s*16 + j (rows 16..127 replicate 0..15)
        idx = nc.declare_dram_parameter("idx", [P, IDX_COLS], i16, isOutput=False)
    else:
        # idx[p, t] = rel id of local row p*T + t, int32
        idx = nc.declare_dram_parameter("idx", [P, T], i32, isOutput=False)
    ub = nc.declare_dram_parameter("ub", [ROWS], f32, isOutput=False)
    lb = nc.declare_dram_parameter("lb", [ROWS], f32, isOutput=False)
    table = nc.declare_dram_parameter("table", [NUM_REL, D], tdt, isOutput=False)
    out = nc.declare_dram_parameter("out", [ROWS], f32, isOutput=True)

    h_v = h[:].rearrange("(p t) d -> p t d", p=P)
    t_v = t_[:].rearrange("(p t) d -> p t d", p=P)
    ub_v = ub[:].rearrange("(p t) -> p t", p=P)
    lb_v = lb[:].rearrange("(p t) -> p t", p=P)
    out_v = out[:].rearrange("(p t) -> p t", p=P)

    with tile.TileContext(nc) as tc:
        with (
            tc.tile_pool(name="io", bufs=1) as io_pool,
            tc.tile_pool(name="stream", bufs=3) as spool,
            tc.tile_pool(name="rpool", bufs=3) as rpool,
            tc.tile_pool(name="qpool", bufs=2) as qpool,
        ):
            if GATHER_MODE == "dma_gather":
                idxs_all = io_pool.tile([P, IDX_COLS], i16)
            else:
                idxs_all = io_pool.tile([P, T], i32)
            nc.sync.dma_start(out=idxs_all[:], in_=idx[:])
            scores = io_pool.tile([P, T], f32)

            for c in range(NCH):
                t0 = c * G
                htile = spool.tile([P, G * D], sdt, tag="h")
                ttile = spool.tile([P, G * D], sdt, tag="t")
                rtile = rpool.tile([P, G * D], tdt, tag="r")
                nc.sync.dma_start(
                    out=htile[:].rearrange("p (g d) -> p g d", g=G),
                    in_=h_v[:, t0 : t0 + G, :],
                )
                nc.sync.dma_start(
                    out=ttile[:].rearrange("p (g d) -> p g d", g=G),
                    in_=t_v[:, t0 : t0 + G, :],
                )
                if GATHER_MODE == "dma_gather":
                    for s in range(NSUB):
                        k = c * NSUB + s
                        k0 = k * (GATHER_IDX // 16)
                        nc.gpsimd.dma_gather(
                            out_ap=rtile[
                                :, s * GB * D : (s + 1) * GB * D
                            ].rearrange("p (g d) -> p g d", g=GB),
                            in_ap=table[:],
                            idxs_ap=idxs_all[:, k0 : k0 + GATHER_IDX // 16],
                            num_idxs=GATHER_IDX,
                            num_idxs_reg=GATHER_IDX,
                            elem_size=D,
                        )
                elif GATHER_MODE == "indirect":
                    nc.gpsimd.indirect_dma_start(
                        out=rtile[:].rearrange("p (g d) -> p g d", g=G),
                        out_offset=None,
                        in_=table[:],
                        in_offset=bass.IndirectOffsetOnAxis(
                            ap=idxs_all[:, t0 : t0 + G], axis=0
                        ),
                    )
                else:
                    for gi in range(G):
                        nc.gpsimd.indirect_dma_start(
                            out=rtile[:, gi * D : (gi + 1) * D],
                            out_offset=None,
                            in_=table[:],
                            in_offset=bass.IndirectOffsetOnAxis(
                                ap=idxs_all[:, t0 + gi : t0 + gi + 1], axis=0
                            ),
                        )
                q = qpool.tile([P, G * D], bf16, tag="q")
                rq = qpool.tile([P, G * D], bf16, tag="rq")
                nc.vector.tensor_tensor(
                    out=q[:], in0=htile[:], in1=ttile[:], op=mult
                )
                nc.vector.tensor_tensor(
                    out=rq[:], in0=q[:], in1=rtile[:], op=mult
                )
                nc.vector.tensor_reduce(
                    out=scores[:, t0 : t0 + G],
                    in_=rq[:].rearrange("p (g d) -> p g d", g=G),
                    axis=mybir.AxisListType.X,
                    op=add,
                )

            # tail: clamp + sigmoid + store
            ubt = io_pool.tile([P, T], f32)
            lbt = io_pool.tile([P, T], f32)
            nc.sync.dma_start(out=ubt[:], in_=ub_v)
            nc.sync.dma_start(out=lbt[:], in_=lb_v)
            clip1 = io_pool.tile([P, T], f32)
            clip2 = io_pool.tile([P, T], f32)
            nc.vector.tensor_tensor(
                out=clip1[:], in0=scores[:], in1=ubt[:], op=mybir.AluOpType.min
            )
            nc.vector.tensor_tensor(
                out=clip2[:], in0=clip1[:], in1=lbt[:], op=mybir.AluOpType.max
            )
            sig = io_pool.tile([P, T], f32)
            nc.scalar.activation(
                out=sig[:], in_=clip2[:], func=mybir.ActivationFunctionType.Sigmoid
            )
            nc.sync.dma_start(out=out_v, in_=sig[:])

    nc.compile()
    return nc


def make_in_maps(inputs: dict, stream_dt: str = STREAM_DT, table_dt: str = TABLE_DT):
    np_sdt = _np_dt(stream_dt)
    np_tdt = _np_dt(table_dt)

    head = np.asarray(inputs["head_embeds"], dtype=np.float32).astype(np_sdt)
    tail = np.asarray(inputs["tail_embeds"], dtype=np.float32).astype(np_sdt)
    rel_ids = np.asarray(inputs["rel_ids"]).astype(np.int32)
    lower = np.asarray(inputs["lower_bound"], dtype=np.float32)
    upper = np.asarray(inputs["upper_bound"], dtype=np.float32)
    table = np.asarray(inputs["rel_embeds"], dtype=np.float32).astype(np_tdt)

    pos_inf = np.full(ROWS, np.inf, dtype=np.float32)
    neg_inf = np.full(ROWS, -np.inf, dtype=np.float32)

    in_maps = []
    for c in range(NCORES):
        lo = c * ROWS
        hi = lo + ROWS
        if hi <= N_POS:
            ub_c = upper[lo:hi]
            lb_c = neg_inf
        else:
            assert lo >= N_POS
            ub_c = pos_inf
            lb_c = lower[lo - N_POS : hi - N_POS]
        # local row r = p*T + t
        rel_local = rel_ids[lo:hi].reshape(P, T)
        if GATHER_MODE == "dma_gather":
            # gather element i of call k is row (p = i%128, t = k*GB + i//128),
            # stored at idx[i%16, k*(GATHER_IDX//16) + i//16]
            blocks = []
            for k in range(T // GB):
                flat = rel_local[:, k * GB : (k + 1) * GB].T.reshape(-1)
                blocks.append(flat.reshape(-1, 16).T)  # [16, GATHER_IDX//16]
            idx16 = np.hstack(blocks)  # [16, IDX_COLS]
            idx_full = np.tile(idx16, (P // 16, 1)).astype(np.int16)
        else:
            idx_full = rel_local.astype(np.int32)  # [128, T]
        in_maps.append(
            {
                "h": np.ascontiguousarray(head[lo:hi]),
                "t": np.ascontiguousarray(tail[lo:hi]),
                "idx": np.ascontiguousarray(idx_full),
                "ub": np.ascontiguousarray(ub_c),
                "lb": np.ascontiguousarray(lb_c),
                "table": table,
            }
        )
    return in_maps


def kernel(**inputs) -> np.ndarray:
    nc = build_program(STREAM_DT, TABLE_DT)
    in_maps = make_in_maps(inputs, STREAM_DT, TABLE_DT)
    res = run_bass_kernel_spmd(nc, in_maps, list(range(NCORES)))
    return np.concatenate([res.results[c]["out"] for c in range(NCORES)])


# revision 19
# speedup vs baseline: 2.1877x; 1.6719x over previous
"""DistMult scoring kernel for Trainium2 (8 NeuronCores, Bass/Tile).

reference computation:
    rel = rel_embeds[rel_ids]                      # [B, D] gather
    scores = sum(head * rel * tail, axis=-1)       # [B]
    pos = min(scores[:n_pos], upper_bound)
    neg = max(scores[n_pos:], lower_bound)
    out = sigmoid(concat(pos, neg))

Sharding: data-parallel over B. Core c owns rows [c*65536, (c+1)*65536).
The pos/neg split (131072 = 2*65536) falls on a core boundary; cores get
+/-inf padded bounds so each computes out = sigmoid(max(min(s, ub), lb)).

Measured-hardware design notes:
  * SWDGE descriptor generation costs ~10ns/descriptor on the GpSimd Q7,
    so per-row rel gathers (65536 descs/core) are hopeless. Instead the host
    permutes rows (a pure layout transform; inverse permutation is applied
    to the output) so rows sharing a rel id sit in blocks of 8 within each
    SBUF partition strip. One dma_gather descriptor then serves 8 rows
    (8704 descs/core ~ 90us), and the multiply reads the gathered [BPC, D]
    tile through a stride-0 broadcast AP.
  * DVE tensor_tensor runs 2 elem/cycle/lane only when all operands are
    16-bit, so h/t stream as fp8 in HBM but are upcast to bf16 during the
    DMA (SWDGE cast-copy); the rel tile is upcast fp8->bf16 on the Scalar
    engine. All elementwise math is then bf16 at 2x.
  * tensor_reduce is hardwired 1 elem/cycle, so the D-reduction is done
    with bf16 halving adds (2x) split between DVE and GpSimd, with only a
    final 32->1 tensor_reduce on DVE.
Numerics: fp8_e4m3 inputs, bf16 products/tree -> l2 err ~1.0e-2 (< 2e-2).
"""

import sys

for _p in ("/opt/trn_rl_repo",):
    if _p not in sys.path:
        sys.path.insert(0, _p)

import numpy as np

import concourse.bacc as bacc
import concourse.bass as bass
import concourse.mybir as mybir
import concourse.tile as tile
from concourse.bass_utils import run_bass_kernel_spmd

N_POS = 131072
N_NEG = 393216
B = N_POS + N_NEG  # 524288
D = 256
NUM_REL = 500
NCORES = 8
ROWS = B // NCORES  # 65536 real rows per core
P = 128

BLK = 8  # rows per rel-uniform block (one gather descriptor serves BLK rows)
TP = 544  # padded strip length: 544*128 = 69632 >= 65536 + worst-case padding
BPC = TP // BLK  # 68 blocks per partition strip
NBLK = P * BPC  # 8704 blocks per core
NPAD = P * TP  # padded rows per core

CH = 32  # t-columns per chunk (stream DMA + compute granularity)
NCHU = TP // CH  # 17 chunks
GIDX = 1024  # indices per dma_gather call (65 descs/lane; ring limit is 128)
IDX_COLS = NBLK // 16  # 544 int16 columns in the idx tensor


def build_program():
    f8 = mybir.dt.float8e4
    bf16 = mybir.dt.bfloat16
    f32 = mybir.dt.float32
    i16 = mybir.dt.int16
    mult = mybir.AluOpType.mult
    add = mybir.AluOpType.add

    nc = bacc.Bacc(
        "TRN2", target_bir_lowering=False, debug=False, num_devices=NCORES
    )
    h = nc.declare_dram_parameter("h", [NPAD, D], f8, isOutput=False)
    t_ = nc.declare_dram_parameter("t", [NPAD, D], f8, isOutput=False)
    idx = nc.declare_dram_parameter("idx", [P, IDX_COLS], i16, isOutput=False)
    ub = nc.declare_dram_parameter("ub", [NPAD], f32, isOutput=False)
    lb = nc.declare_dram_parameter("lb", [NPAD], f32, isOutput=False)
    table = nc.declare_dram_parameter("table", [NUM_REL, D], f8, isOutput=False)
    out = nc.declare_dram_parameter("out", [NPAD], f32, isOutput=True)

    h_v = h[:].rearrange("(p t) d -> p t d", p=P)
    t_v = t_[:].rearrange("(p t) d -> p t d", p=P)
    ub_v = ub[:].rearrange("(p t) -> p t", p=P)
    lb_v = lb[:].rearrange("(p t) -> p t", p=P)
    out_v = out[:].rearrange("(p t) -> p t", p=P)

    with tile.TileContext(nc) as tc:
        with (
            tc.tile_pool(name="io", bufs=1) as io_pool,
            tc.tile_pool(name="stream", bufs=2) as spool,
            tc.tile_pool(name="relp", bufs=2) as rpool,
            tc.tile_pool(name="qp", bufs=1) as qpool,
            tc.tile_pool(name="hv", bufs=2) as hpool,
        ):
            idxs_all = io_pool.tile([P, IDX_COLS], i16)
            nc.sync.dma_start(out=idxs_all[:], in_=idx[:])
            scores = io_pool.tile([P, TP], f32)
            # whole gathered rel table slice for this core, fp8 [128, 68*256]
            rel8 = io_pool.tile([P, BPC * D], f8)

            # 9 gather calls: 8 x 1024 idxs + 1 x 512 idxs
            nblk_done = 0
            while nblk_done < BPC:
                nb = min(GIDX // P, BPC - nblk_done)  # blocks/partition
                nidx = nb * P
                c0 = nblk_done * (P // 16)  # int16 col offset: 8 cols per blk
                nc.gpsimd.dma_gather(
                    out_ap=rel8[
                        :, nblk_done * D : (nblk_done + nb) * D
                    ].rearrange("p (b d) -> p b d", b=nb),
                    in_ap=table[:],
                    idxs_ap=idxs_all[:, c0 : c0 + nidx // 16],
                    num_idxs=nidx,
                    num_idxs_reg=nidx,
                    elem_size=D,
                )
                nblk_done += nb

            for c in range(NCHU):
                t0 = c * CH
                b0 = t0 // BLK  # first block of chunk
                nb = CH // BLK  # blocks per chunk (4)
                htile = spool.tile([P, CH * D], bf16, tag="h")
                ttile = spool.tile([P, CH * D], bf16, tag="t")
                # SWDGE cast-copy: fp8 in DRAM -> bf16 in SBUF
                nc.gpsimd.dma_start(
                    out=htile[:].rearrange("p (g d) -> p g d", g=CH),
                    in_=h_v[:, t0 : t0 + CH, :],
                )
                nc.gpsimd.dma_start(
                    out=ttile[:].rearrange("p (g d) -> p g d", g=CH),
                    in_=t_v[:, t0 : t0 + CH, :],
                )

                # rel chunk: cast fp8 -> bf16 on the Scalar engine
                relb = rpool.tile([P, nb * D], bf16, tag="rb")
                nc.scalar.activation(
                    out=relb[:],
                    in_=rel8[:, b0 * D : (b0 + nb) * D],
                    func=mybir.ActivationFunctionType.Copy,
                )

                q = qpool.tile([P, CH * D], bf16, tag="q")
                rq = qpool.tile([P, CH * D], bf16, tag="rq")
                nc.vector.tensor_tensor(
                    out=q[:], in0=htile[:], in1=ttile[:], op=mult
                )
                # rq = q * rel (rel broadcast x8 along rows of each block)
                nc.vector.tensor_tensor(
                    out=rq[:].rearrange("p (b e d) -> p b e d", b=nb, e=BLK),
                    in0=q[:].rearrange("p (b e d) -> p b e d", b=nb, e=BLK),
                    in1=relb[:]
                    .rearrange("p (b d) -> p b d", b=nb)
                    .unsqueeze(2)
                    .broadcast_to([P, nb, BLK, D]),
                    op=mult,
                )
                # halving tree: 256 -> 128 (DVE) -> 64 -> 32 (GpSimd) -> reduce
                s1 = hpool.tile([P, CH * 128], bf16, tag="s1")
                v = rq[:].rearrange("p (g u x) -> p g u x", g=CH, u=2)
                nc.vector.tensor_tensor(
                    out=s1[:].rearrange("p (g x) -> p g x", g=CH),
                    in0=v[:, :, 0, :],
                    in1=v[:, :, 1, :],
                    op=add,
                )
                s2 = hpool.tile([P, CH * 64], bf16, tag="s2")
                v = s1[:].rearrange("p (g u x) -> p g u x", g=CH, u=2)
                nc.gpsimd.tensor_tensor(
                    out=s2[:].rearrange("p (g x) -> p g x", g=CH),
                    in0=v[:, :, 0, :],
                    in1=v[:, :, 1, :],
                    op=add,
                )
                s3 = hpool.tile([P, CH * 32], bf16, tag="s3")
                v = s2[:].rearrange("p (g u x) -> p g u x", g=CH, u=2)
                nc.vector.tensor_tensor(
                    out=s3[:].rearrange("p (g x) -> p g x", g=CH),
                    in0=v[:, :, 0, :],
                    in1=v[:, :, 1, :],
                    op=add,
                )
                nc.vector.tensor_reduce(
                    out=scores[:, t0 : t0 + CH],
                    in_=s3[:].rearrange("p (g x) -> p g x", g=CH),
                    axis=mybir.AxisListType.X,
                    op=add,
                )

            # tail: clamp + sigmoid + store
            ubt = io_pool.tile([P, TP], f32)
            lbt = io_pool.tile([P, TP], f32)
            nc.sync.dma_start(out=ubt[:], in_=ub_v)
            nc.sync.dma_start(out=lbt[:], in_=lb_v)
            clip1 = io_pool.tile([P, TP], f32)
            clip2 = io_pool.tile([P, TP], f32)
            nc.vector.tensor_tensor(
                out=clip1[:], in0=scores[:], in1=ubt[:], op=mybir.AluOpType.min
            )
            nc.vector.tensor_tensor(
                out=clip2[:], in0=clip1[:], in1=lbt[:], op=mybir.AluOpType.max
            )
            sig = io_pool.tile([P, TP], f32)
            nc.scalar.activation(
                out=sig[:], in_=clip2[:], func=mybir.ActivationFunctionType.Sigmoid
            )
            nc.sync.dma_start(out=out_v, in_=sig[:])

    nc.compile()
    return nc


def _plan_core(rel_local: np.ndarray):
    """Sort rows into rel-uniform blocks of BLK, dealt to 128 strips of BPC.

    Returns (perm, block_rel): perm[NPAD] = original local row index or -1
    (device row p*TP+t = padded slot p*TP+t), block_rel[P, BPC] int16.
    """
    order = np.argsort(rel_local, kind="stable").astype(np.int64)
    c = np.bincount(rel_local, minlength=NUM_REL).astype(np.int64)
    pad_c = ((c + BLK - 1) // BLK) * BLK
    total_pad = int(pad_c.sum())
    assert total_pad <= NPAD, (total_pad, NPAD)
    starts_sorted = np.zeros(NUM_REL, np.int64)
    starts_sorted[1:] = np.cumsum(c)[:-1]
    starts_padded = np.zeros(NUM_REL, np.int64)
    starts_padded[1:] = np.cumsum(pad_c)[:-1]
    cum_pad = np.cumsum(pad_c)

    s = np.arange(total_pad, dtype=np.int64)
    j = np.searchsorted(cum_pad, s, side="right")
    o = s - starts_padded[j]
    valid = o < c[j]
    rows = np.where(
        valid, order[np.minimum(starts_sorted[j] + o, ROWS - 1)], -1
    )
    perm = np.full(NPAD, -1, np.int64)
    perm[:total_pad] = rows
    slot_rel = np.zeros(NPAD, np.int64)
    slot_rel[:total_pad] = j
    block_rel = slot_rel[::BLK].reshape(P, BPC).astype(np.int16)
    return perm, block_rel


def make_in_maps(inputs: dict):
    import ml_dtypes

    f8 = ml_dtypes.float8_e4m3
    head = np.asarray(inputs["head_embeds"], dtype=np.float32).astype(f8)
    tail = np.asarray(inputs["tail_embeds"], dtype=np.float32).astype(f8)
    rel_ids = np.asarray(inputs["rel_ids"]).astype(np.int64)
    lower = np.asarray(inputs["lower_bound"], dtype=np.float32)
    upper = np.asarray(inputs["upper_bound"], dtype=np.float32)
    table = np.asarray(inputs["rel_embeds"], dtype=np.float32).astype(f8)

    in_maps = []
    perms = []
    for core in range(NCORES):
        lo = core * ROWS
        hi = lo + ROWS
        perm, block_rel = _plan_core(rel_ids[lo:hi])
        perms.append(perm)

        pvalid = perm >= 0
        psafe = np.where(pvalid, perm, 0)
        h_c = np.where(pvalid[:, None], head[lo:hi][psafe], f8(0))
        t_c = np.where(pvalid[:, None], tail[lo:hi][psafe], f8(0))
        if hi <= N_POS:
            ub_c = np.where(pvalid, upper[lo:hi][psafe], np.inf).astype(
                np.float32
            )
            lb_c = np.full(NPAD, -np.inf, np.float32)
        else:
            assert lo >= N_POS
            ub_c = np.full(NPAD, np.inf, np.float32)
            lb_c = np.where(
                pvalid, lower[lo - N_POS : hi - N_POS][psafe], -np.inf
            ).astype(np.float32)

        # dma_gather idx layout: call k covers nb blocks/partition starting
        # at block b0; element i -> (p=i%128, blk=b0+i//128), stored at
        # int16 [i%16, col0 + i//16]
        blocks16 = []
        nblk_done = 0
        while nblk_done < BPC:
            nb = min(GIDX // P, BPC - nblk_done)
            flat = block_rel[:, nblk_done : nblk_done + nb].T.reshape(-1)
            blocks16.append(flat.reshape(-1, 16).T)
            nblk_done += nb
        idx16 = np.hstack(blocks16)  # [16, IDX_COLS]
        idx_full = np.tile(idx16, (P // 16, 1)).astype(np.int16)

        in_maps.append(
            {
                "h": np.ascontiguousarray(h_c),
                "t": np.ascontiguousarray(t_c),
                "idx": np.ascontiguousarray(idx_full),
                "ub": np.ascontiguousarray(ub_c),
                "lb": np.ascontiguousarray(lb_c),
                "table": table,
            }
        )
    return in_maps, perms


def unpermute(results: list, perms: list) -> np.ndarray:
    out = np.empty(B, dtype=np.float32)
    for core in range(NCORES):
        dev = results[core]  # [NPAD] f32 in padded-sorted order
        perm = perms[core]
        pvalid = perm >= 0
        seg = out[core * ROWS : (core + 1) * ROWS]
        seg[perm[pvalid]] = dev[pvalid]
    return out


def kernel(**inputs) -> np.ndarray:
    nc = build_program()
    in_maps, perms = make_in_maps(inputs)
    res = run_bass_kernel_spmd(nc, in_maps, list(range(NCORES)))
    return unpermute([res.results[c]["out"] for c in range(NCORES)], perms)
